# revision 46
# baseline (speedup 1.0000x reference)
"""Two-layer single-head GAT on Trainium2 (8 NeuronCores, Bass/Tile).

Strategy (graph-parallel over dst nodes):
  - Relabel nodes into "slots": 8 cores x NW windows x 128 slots. Nodes are
    assigned to cores balanced by degree (serpentine over degree-sorted
    order), then packed into windows (<=128 nodes, capped total in-degree,
    capped per-src-bucket in-degree).
  - Per layer, each core builds its shard of a node table
    row[n] = [h(64) bf16 | 1.0 | a_s_hi | a_s_lo] via matmuls (h = x@W,
    a_s = x@(W@att_src)), then the 8 shards are AllGathered so every core
    holds the full table in its DRAM.
  - Edges live on the core that owns their dst. Per-edge rows h[src] are
    fetched with dma_gather (int16 idx => the table is read in 4 bucket
    slices of <=32768 rows; bucket(src) = src_core//2).
  - One-hot scatter masks M_T[e, d, t] = (d == dst_local[e,t]) are built on
    DVE with a repeated-iota constant so both operands are packed bf16
    (2x DVE mode). Per-edge a_d[dst] comes from an "a_d sweep" timed to
    fill each AllGather window: per group the mask is built once, its used
    tiles are transposed on PE (identity permutation), copied to SBUF, and
    a near-free PE matvec against the per-window a_d column (computed
    slot-on-partition in phase A) yields a_d per edge.
  - ex = exp(leaky_relu(a_s+a_d)); softmax max-subtraction is skipped
    (scores are O(10), exp stays in fp32 range; alpha is identical).
  - Aggregation: per window PSUM accumulates lhsT=[ex*h | ex] (128e x 65)
    @ rhs=M_T[:, :, t] (128e x 128d) -> [65 x 128d]; per-group epilogue
    divides by the ex-sum row, adds bias (and relu between layers).
  - Layer-2 phase A (table build) is interleaved into layer-1's edge phase;
    the layer-2 table is AllGathered in TWO halves (windows < NWA vs the
    rest) with layer-2-specific src buckets (half, half-of-half-table):
    the A-half collective hides under layer-1's edge phase, and during the
    B-half collective the A-bucket edges are gathered, scored and
    pre-aggregated into a DRAM stash (reusing the sweep's mask tiles);
    the B pass then only processes B buckets and merges the stash.
Outputs are written transposed ([64, slots]) and un-permuted on the host.
"""

import numpy as np
import ml_dtypes

BF16 = ml_dtypes.bfloat16

NCORES = 8
P = 128
D = 64
NEG_SLOPE = 0.2
EPS = 1e-16

EWCAP = 2040      # max total in-degree per window
NODECAP = 128     # max nodes per window
TPBMAX = 6        # tiles per (window, bucket); bucket in-degree cap = 128*TPBMAX
GRP = 3           # windows per gather group (CALL=GRP*TPB*128 must stay
                  # under ~2500: one dma_gather's walrus sub-DMA semaphore
                  # arithmetic overflows a 16-bit ISA field beyond that)

_CACHE = {}


def _preprocess(x, edge_index):
    """Host-side partitioning/indexing. Returns per-core input arrays + meta."""
    N = x.shape[0]
    E = edge_index.shape[1]
    src = edge_index[0].astype(np.int64)
    dst = edge_index[1].astype(np.int64)

    deg = np.bincount(dst, minlength=N)

    # --- assign nodes to cores: serpentine over degree-sorted order ---
    order = np.argsort(-deg, kind="stable")
    core_of_node = np.empty(N, np.int32)
    pat = np.concatenate([np.arange(NCORES), np.arange(NCORES)[::-1]])
    core_of_node[order] = pat[np.arange(N) % (2 * NCORES)]

    bucket_of_node = core_of_node // 2  # 4 buckets of 2 cores each

    # per-node in-degree per src bucket
    deg_b = np.zeros((N, 4), np.int64)
    for b in range(4):
        m = bucket_of_node[src] == b
        deg_b[:, b] = np.bincount(dst[m], minlength=N)

    # --- pack windows per core ---
    bcap = P * TPBMAX
    windows = [[] for _ in range(NCORES)]  # list of lists of node ids
    for c in range(NCORES):
        nodes_c = order[core_of_node[order] == c]  # degree-sorted
        cur, cur_deg, cur_b = [], 0, np.zeros(4, np.int64)
        for n in nodes_c:
            d_n = deg[n]
            db_n = deg_b[n]
            if cur and (len(cur) >= NODECAP or cur_deg + d_n > EWCAP
                        or np.any(cur_b + db_n > bcap)):
                windows[c].append(cur)
                cur, cur_deg, cur_b = [], 0, np.zeros(4, np.int64)
            cur.append(n)
            cur_deg += d_n
            cur_b = cur_b + db_n
        if cur:
            windows[c].append(cur)

    nw_real = max(len(w) for w in windows)
    NG = -(-nw_real // GRP)
    NW = NG * GRP
    SLOTS_PC = NW * P
    NSLOT = NCORES * SLOTS_PC
    BSZ = NSLOT // 4
    assert BSZ <= 32768, f"int16 gather range exceeded: BSZ={BSZ}"

    # --- slot assignment ---
    slot_of_node = np.full(N, -1, np.int64)
    for c in range(NCORES):
        for w, wl in enumerate(windows[c]):
            base = c * SLOTS_PC + w * P
            slot_of_node[np.asarray(wl, np.int64)] = base + np.arange(len(wl))
    assert (slot_of_node >= 0).all()

    sslot = slot_of_node[src]
    dslot = slot_of_node[dst]
    ecore = (dslot // SLOTS_PC).astype(np.int32)
    ew = (dslot % SLOTS_PC) // P          # window within core
    eb = (sslot // BSZ).astype(np.int32)  # src bucket (layer 1)
    edloc = (dslot % P).astype(np.int32)  # dst slot within window
    esidx = (sslot % BSZ).astype(np.int64)  # idx within bucket slice

    # layer-2 table is AllGathered in two halves (windows < NWA vs rest),
    # so its src buckets are (half, half-of-half-table)
    NGA = (NG + 1) // 2
    NWA = NGA * GRP
    NWB = NW - NWA
    RA, RB = NWA * P, NWB * P
    BSZ2A, BSZ2B = NCORES * RA // 2, NCORES * RB // 2
    assert BSZ2A <= 32768 and BSZ2B <= 32768
    sc_ = sslot // SLOTS_PC
    sw = (sslot % SLOTS_PC) // P
    si = sslot % P
    inA = sw < NWA
    row2 = np.where(inA, sc_ * RA + sw * P + si,
                    sc_ * RB + (sw - NWA) * P + si)
    eb2 = np.where(inA, (row2 >= BSZ2A).astype(np.int64),
                   2 + (row2 >= BSZ2B).astype(np.int64)).astype(np.int32)
    esidx2 = np.where(inA, row2 % BSZ2A, row2 % BSZ2B)

    key = ((ecore.astype(np.int64) * NW + ew) * 4 + eb)
    key2 = ((ecore.astype(np.int64) * NW + ew) * 4 + eb2)
    cnt = np.bincount(key, minlength=NCORES * NW * 4).reshape(NCORES, NW, 4)
    cnt2 = np.bincount(key2, minlength=NCORES * NW * 4).reshape(NCORES, NW, 4)
    tiles_used = -(-cnt.max(axis=0) // P)   # [NW, 4], same for all cores
    tiles_used2 = -(-cnt2.max(axis=0) // P)
    TPB = int(max(-(-cnt.max() // P), -(-cnt2.max() // P)))
    assert TPB <= TPBMAX, f"bucket cap violated: TPB={TPB}"
    CW = TPB * P                      # slots per (window, bucket)
    CALL = GRP * CW                   # idxs per dma_gather call
    NCOLS = 4 * GRP * TPB             # dstloc cols per group

    def edge_tables(key_l, esidx_l, tu):
        # windows are packed back-to-back inside each (group, bucket) call
        # at cumulative tile offsets; the call's num_idxs shrinks to the
        # used-tile count (cut gather descriptors ~11%)
        cum = np.zeros((NG, 4, GRP + 1), np.int64)
        for g in range(NG):
            for b in range(4):
                for wl in range(GRP):
                    cum[g, b, wl + 1] = cum[g, b, wl] + tu[g * GRP + wl][b]
        eorder = np.argsort(key_l, kind="stable")
        key_s = key_l[eorder]
        gidx = np.zeros((NCORES, NG, 4, CALL), np.int16)
        dloc = np.full((NCORES, NG, 4, GRP * TPB, P), 300.0, np.float32)
        starts = np.zeros(NCORES * NW * 4 + 1, np.int64)
        np.cumsum(np.bincount(key_s, minlength=NCORES * NW * 4),
                  out=starts[1:])
        esidx_s = esidx_l[eorder]
        edloc_s = edloc[eorder]
        for c in range(NCORES):
            for w in range(NW):
                g, wl = divmod(w, GRP)
                for b in range(4):
                    k = (c * NW + w) * 4 + b
                    lo, hi = starts[k], starts[k + 1]
                    n = hi - lo
                    if n == 0:
                        continue
                    off = int(cum[g, b, wl]) * P
                    gidx[c, g, b, off:off + n] = \
                        esidx_s[lo:hi].astype(np.int16)
                    tt = (np.arange(n) // P) + int(cum[g, b, wl])
                    pp = np.arange(n) % P
                    dloc[c, g, b, tt, pp] = edloc_s[lo:hi].astype(np.float32)
        # wrap-16 + replicate to 128 partitions
        g16 = gidx.reshape(NCORES, NG * 4, CALL // 16, 16) \
            .transpose(0, 1, 3, 2)
        g128 = np.tile(g16, (1, 1, 8, 1)) \
            .reshape(NCORES, NG * 4 * 128, CALL // 16)
        dl = dloc.transpose(0, 4, 1, 2, 3).reshape(NCORES, P, NG * NCOLS)
        return g128, np.ascontiguousarray(dl).astype(BF16)

    g128, dl = edge_tables(key, esidx, tiles_used)
    g128_2, dl2 = edge_tables(key2, esidx2, tiles_used2)

    # permuted, transposed x per core
    node_of_slot = np.full(NSLOT, -1, np.int64)
    node_of_slot[slot_of_node] = np.arange(N)
    xT = np.zeros((NCORES, D, SLOTS_PC), BF16)
    for c in range(NCORES):
        sl = node_of_slot[c * SLOTS_PC:(c + 1) * SLOTS_PC]
        valid = sl >= 0
        blk = np.zeros((SLOTS_PC, D), np.float32)
        blk[valid] = x[sl[valid]]
        xT[c] = blk.T.astype(BF16)

    meta = dict(NW=NW, NG=NG, TPB=TPB, CW=CW, CALL=CALL, NCOLS=NCOLS,
                SLOTS_PC=SLOTS_PC, NSLOT=NSLOT, BSZ=BSZ, N=N,
                NWA=NWA, NWB=NWB, BSZ2A=BSZ2A, BSZ2B=BSZ2B,
                tiles_used=tuple(map(tuple, tiles_used)),
                tiles_used2=tuple(map(tuple, tiles_used2)))
    percore = dict(xT=xT, gidx=g128, dstloc=dl, gidx2=g128_2, dstloc2=dl2)
    return meta, percore, node_of_slot


def _build_program(meta):
    import concourse.bacc as bacc
    import concourse.tile as tile
    from concourse import mybir

    F32, BF, I16 = mybir.dt.float32, mybir.dt.bfloat16, mybir.dt.int16
    Alu = mybir.AluOpType
    Act = mybir.ActivationFunctionType

    NW, NG, TPB = meta["NW"], meta["NG"], meta["TPB"]
    tiles_used = meta["tiles_used"]
    tiles_used2 = meta["tiles_used2"]
    CALL, NCOLS = meta["CALL"], meta["NCOLS"]
    SLOTS_PC, NSLOT, BSZ = meta["SLOTS_PC"], meta["NSLOT"], meta["BSZ"]
    NWA, NWB = meta["NWA"], meta["NWB"]
    BSZ2A, BSZ2B = meta["BSZ2A"], meta["BSZ2B"]
    GT = GRP * TPB
    EPW = 384  # epilogue batch width: GRP windows x 128 slots

    nc = bacc.Bacc("TRN2", target_bir_lowering=False, debug=False,
                   num_devices=NCORES)

    xT_d = nc.dram_tensor("xT", [D, SLOTS_PC], BF, kind="ExternalInput")
    gidx_d = nc.dram_tensor("gidx", [NG * 4 * 128, CALL // 16], I16,
                            kind="ExternalInput")
    gidx2_d = nc.dram_tensor("gidx2", [NG * 4 * 128, CALL // 16], I16,
                             kind="ExternalInput")
    dstloc_d = nc.dram_tensor("dstloc", [P, NG * NCOLS], BF,
                              kind="ExternalInput")
    dstloc2_d = nc.dram_tensor("dstloc2", [P, NG * NCOLS], BF,
                               kind="ExternalInput")
    w1cat_d = nc.dram_tensor("w1cat", [D, 65], BF, kind="ExternalInput")
    w2cat_d = nc.dram_tensor("w2cat", [D, 65], BF, kind="ExternalInput")
    wd1_d = nc.dram_tensor("wd1rep", [D, 128], BF, kind="ExternalInput")
    wd2_d = nc.dram_tensor("wd2rep", [D, 128], BF, kind="ExternalInput")
    b1_d = nc.dram_tensor("b1", [D, 1], F32, kind="ExternalInput")
    b2_d = nc.dram_tensor("b2", [D, 1], F32, kind="ExternalInput")
    ones1_d = nc.dram_tensor("ones1", [1, D], F32, kind="ExternalInput")
    out_d = nc.dram_tensor("out2T", [D, SLOTS_PC], F32, kind="ExternalOutput")

    shard1 = nc.dram_tensor("shard1", [SLOTS_PC, 128], BF)
    shard2a = nc.dram_tensor("shard2a", [NWA * P, 128], BF)
    shard2b = nc.dram_tensor("shard2b", [NWB * P, 128], BF)
    tbl1 = nc.dram_tensor("tbl1", [NSLOT, 128], BF, addr_space="Shared")
    tbl2a = nc.dram_tensor("tbl2a", [2 * BSZ2A, 128], BF,
                           addr_space="Shared")
    tbl2b = nc.dram_tensor("tbl2b", [2 * BSZ2B, 128], BF,
                           addr_space="Shared")
    stash_d = nc.dram_tensor("stash", [NG * 65, GRP * P], F32)

    with tile.TileContext(nc) as tc:
        import contextlib
        stack = contextlib.ExitStack()
        with stack:
            const = stack.enter_context(tc.tile_pool(name="const", bufs=1))
            small = stack.enter_context(tc.tile_pool(name="small", bufs=3))
            vp = stack.enter_context(tc.tile_pool(name="vp", bufs=3))
            mp = stack.enter_context(tc.tile_pool(name="mp", bufs=2))
            swp = stack.enter_context(tc.tile_pool(name="swp", bufs=3))
            msp = stack.enter_context(tc.tile_pool(name="msp", bufs=2))
            sc = stack.enter_context(tc.tile_pool(name="sc", bufs=3))
            ip = stack.enter_context(tc.tile_pool(name="ip", bufs=6))
            ep = stack.enter_context(tc.tile_pool(name="ep", bufs=2))
            psA = stack.enter_context(tc.tile_pool(name="psA", bufs=2, space="PSUM"))
            psC = stack.enter_context(tc.tile_pool(name="psC", bufs=2, space="PSUM"))
            psD = stack.enter_context(tc.tile_pool(name="psD", bufs=1, space="PSUM"))
            psT = stack.enter_context(tc.tile_pool(name="psT", bufs=2, space="PSUM"))
            psF = stack.enter_context(tc.tile_pool(name="psF", bufs=1, space="PSUM"))

            # constants
            iota_b = const.tile([P, 128], BF)
            iotar = const.tile([P, 128 * GT], BF)
            identity = const.tile([P, 128], BF)
            zero128 = const.tile([P, 128], BF)
            nc.gpsimd.memset(zero128[:], 0)
            with tc.tile_pool(name="iotatmp", bufs=1) as iotatmp:
                iota_i = iotatmp.tile([P, 128], I16)
                nc.gpsimd.iota(iota_i[:], pattern=[[1, 128]], base=0,
                               channel_multiplier=0)
                nc.vector.tensor_copy(iota_b[:], iota_i[:])
                # repeated iota: col = d*GT + t -> value d (for one-hot
                # builds with both operands packed => 2x DVE mode)
                iotar_i = iotatmp.tile([P, 128 * GT], I16)
                nc.gpsimd.iota(iotar_i[:], pattern=[[1, 128], [0, GT]],
                               base=0, channel_multiplier=0)
                nc.vector.tensor_copy(iotar[:], iotar_i[:])
                # per-partition iota column -> identity permutation matrix
                # (rhs of the PE transpose used in the a_d sweep)
                iotap_i = iotatmp.tile([P, 1], I16)
                nc.gpsimd.iota(iotap_i[:], pattern=[[0, 1]], base=0,
                               channel_multiplier=1)
                iotap = iotatmp.tile([P, 1], BF)
                nc.vector.tensor_copy(iotap[:], iotap_i[:])
                nc.vector.tensor_tensor(
                    out=identity[:],
                    in0=iotap[:].to_broadcast([P, 128]),
                    in1=iota_b[:], op=Alu.is_equal)
            ones1 = const.tile([1, D], F32)
            nc.sync.dma_start(ones1[:], ones1_d.ap()[:])
            w1cat = const.tile([D, 65], BF)
            nc.sync.dma_start(w1cat[:], w1cat_d.ap()[:])
            w2cat = const.tile([D, 65], BF)
            nc.sync.dma_start(w2cat[:], w2cat_d.ap()[:])
            wd1 = const.tile([D, 128], BF)
            nc.sync.dma_start(wd1[:], wd1_d.ap()[:])
            wd2 = const.tile([D, 128], BF)
            nc.sync.dma_start(wd2[:], wd2_d.ap()[:])
            b1 = const.tile([D, 1], F32)
            nc.sync.dma_start(b1[:], b1_d.ap()[:])
            b2 = const.tile([D, 1], F32)
            nc.sync.dma_start(b2[:], b2_d.ap()[:])

            # resident across layers
            x2T = const.tile([D, SLOTS_PC], BF)
            # a_d per dst slot, transposed: column w holds a_d of window w's
            # 128 slots along partitions (matvec rhs in the a_d sweep)
            adcolT = const.tile([P, NW], BF)
            adpe_d = const.tile([P, NG * NCOLS], F32)
            # dstloc table buffer: loaded with layer-1's layout now,
            # reloaded with layer-2's (different buckets) after layer 1
            dstl_all = const.tile([P, NG * NCOLS], BF)
            nc.sync.dma_start(dstl_all[:], dstloc_d.ap()[:, :])

            for i in range(3):
                vs0 = vp.tile([P, 4, GT, 128], BF, tag="vslab")
                nc.gpsimd.memset(vs0[:], 0)

            # tblrow pool buffers get their constant-1 column (position 64)
            # written once; later phase-A writes never touch that column.
            tbl_tiles = []
            for i in range(3):
                tr = small.tile([P, GRP, 67], BF, tag="tblrow")
                nc.gpsimd.memset(tr[:, :, 64:65], 1.0)
                tbl_tiles.append(tr)

            def phase_a_grp(layer, w0, lhs_list):
                """Table rows + replicated-a_d for GRP consecutive windows,
                with one batched shard write. During the edge phase (layer 1
                interleaved) the copies run on Activation (idle there);
                standalone phase A balances them onto DVE."""
                wcat = w1cat if layer == 0 else w2cat
                wdcol = wd1 if layer == 0 else wd2
                tbuf = small.tile([P, GRP, 67], BF, tag="tblrow")
                psab = psA.tile([P, GRP, 66], F32)
                for k in range(GRP):
                    lhs = lhs_list[k]
                    nc.tensor.matmul(psab[:, k, 0:65], lhsT=lhs, rhs=wcat[:],
                                     start=True, stop=True)
                    # a_d of this window's slots, slots on partitions
                    nc.tensor.matmul(psab[:, k, 65:66], lhsT=lhs,
                                     rhs=wdcol[:, 0:1], start=True, stop=True)
                    if layer == 0:
                        nc.vector.tensor_copy(tbuf[:, k, 0:64],
                                              psab[:, k, 0:64])
                        nc.vector.tensor_copy(tbuf[:, k, 65:66],
                                              psab[:, k, 64:65])
                    else:
                        nc.scalar.copy(tbuf[:, k, 0:64], psab[:, k, 0:64])
                        nc.scalar.copy(tbuf[:, k, 65:66], psab[:, k, 64:65])
                    # a_s_lo residual for extra precision
                    nc.vector.tensor_tensor(out=tbuf[:, k, 66:67],
                                            in0=psab[:, k, 64:65],
                                            in1=tbuf[:, k, 65:66],
                                            op=Alu.subtract)
                if layer == 0:
                    nc.vector.tensor_copy(adcolT[:, w0:w0 + GRP],
                                          psab[:, :, 65])
                else:
                    nc.scalar.copy(adcolT[:, w0:w0 + GRP], psab[:, :, 65])
                if layer == 0:
                    sh, r0 = shard1, w0 * P
                elif w0 < NWA:
                    sh, r0 = shard2a, w0 * P
                else:
                    sh, r0 = shard2b, (w0 - NWA) * P
                nc.sync.dma_start(
                    sh.ap()[r0:r0 + GRP * P, 0:67]
                    .rearrange("(k p) c -> p k c", k=GRP),
                    tbuf[:])

            # ---- layer-0 phase A (x loaded in 12-window chunks) ----
            XC = 9
            for w0 in range(0, NW, XC):
                xtc = small.tile([D, XC * P], BF, tag="xtc")
                hi = min(NW, w0 + XC)
                nc.sync.dma_start(xtc[:, 0:(hi - w0) * P],
                                  xT_d.ap()[:, w0 * P:hi * P])
                for g0 in range(w0, hi, GRP):
                    phase_a_grp(0, g0, [
                        xtc[:, (g0 + k - w0) * P:(g0 + k - w0 + 1) * P]
                        for k in range(GRP)])

            iotar_v = iotar[:].rearrange("p (d t) -> p d t", t=GT)

            def make_cums(tu):
                cs = []
                for g in range(NG):
                    row = []
                    for b in range(4):
                        c = [0]
                        for wl in range(GRP):
                            c.append(c[-1] + tu[g * GRP + wl][b])
                        row.append(c)
                    cs.append(row)
                return cs

            cums1 = make_cums(tiles_used)
            cums2 = make_cums(tiles_used2)

            def load_idx(layer, g, b0, nb):
                """Start DMA of gather index rows for buckets [b0,b0+nb)."""
                gd = gidx_d if layer == 0 else gidx2_d
                idxt = ip.tile([128, nb, CALL // 16], I16, tag=f"idxt{nb}")
                r0 = g * 4 * 128 + b0 * 128
                nc.sync.dma_start(
                    idxt[:],
                    gd.ap()[r0:r0 + nb * 128, :]
                    .rearrange("(b p) c -> p b c", b=nb))
                return idxt

            def bucket_src(layer, b):
                if layer == 0:
                    return tbl1.ap()[b * BSZ:(b + 1) * BSZ, :]
                if b == 0:
                    return tbl2a.ap()[0:BSZ2A, :]
                if b == 1:
                    return tbl2a.ap()[BSZ2A:2 * BSZ2A, :]
                if b == 2:
                    return tbl2b.ap()[0:BSZ2B, :]
                return tbl2b.ap()[BSZ2B:2 * BSZ2B, :]

            def head(layer, g, idxt, b0, nb, dstl, mask=None):
                """Gather launch + one-hot build, buckets [b0, b0+nb)."""
                vslab = vp.tile([P, 4, GT, 128], BF, tag="vslab")
                cums = cums1 if layer == 0 else cums2
                for j in range(nb):
                    b = b0 + j
                    nt = cums[g][b][GRP]
                    if nt == 0:
                        continue
                    nc.gpsimd.dma_gather(
                        out_ap=vslab[:, b, 0:nt, :],
                        in_ap=bucket_src(layer, b),
                        idxs_ap=idxt[:, j, 0:nt * 8], num_idxs=nt * P,
                        num_idxs_reg=nt * P,
                        elem_size=128, single_packet=False)
                if mask is not None:  # reuse the sweep's mask tile
                    return vslab, mask
                mslab = mp.tile([P, 4, 128, GT], BF, tag="mslab")
                c0 = g * NCOLS + b0 * GT
                dstl_v = dstl[:, c0:c0 + nb * GT] \
                    .rearrange("p (b t) -> p b t", b=nb)
                nc.vector.tensor_tensor(
                    out=mslab[:, b0:b0 + nb, :, :],
                    in0=iotar_v[:, None, :, :]
                    .to_broadcast([P, nb, 128, GT]),
                    in1=dstl_v[:, :, None, :]
                    .to_broadcast([P, nb, 128, GT]),
                    op=Alu.is_equal)
                return vslab, mslab

            def score_v(layer, g, st, b0, nb):
                """ex = exp(lrelu(a_s+a_d)); V' = [h|1]*ex, bucket subset."""
                vslab, mslab = st
                c0 = g * NCOLS + b0 * GT
                wcols = nb * GT
                as_t = sc.tile([P, wcols], F32, tag=f"as{nb}{b0}")
                nc.vector.tensor_tensor(
                    out=as_t[:].rearrange("p (b t) -> p b t", b=nb),
                    in0=vslab[:, b0:b0 + nb, :, 65],
                    in1=vslab[:, b0:b0 + nb, :, 66],
                    op=Alu.add)
                nc.vector.tensor_tensor(out=as_t[:], in0=as_t[:],
                                        in1=adpe_d[:, c0:c0 + wcols],
                                        op=Alu.add)
                lr = sc.tile([P, wcols], F32, tag=f"lr{nb}{b0}")
                nc.vector.scalar_tensor_tensor(
                    out=lr[:], in0=as_t[:], scalar=NEG_SLOPE,
                    in1=as_t[:], op0=Alu.mult, op1=Alu.max)
                ex = sc.tile([P, wcols], F32, tag=f"ex{nb}{b0}")
                nc.scalar.activation(ex[:], lr[:], Act.Exp)
                ex_v = ex[:].rearrange("p (b t) -> p b t", b=nb)
                for j in range(nb):
                    b = b0 + j
                    nc.vector.tensor_tensor(
                        out=vslab[:, b, :, 0:65], in0=vslab[:, b, :, 0:65],
                        in1=ex_v[:, j, :, None].to_broadcast([P, GT, 65]),
                        op=Alu.mult)

            def agg_win(st, cums, g, wl, brange):
                """Accumulate used tiles of one window into a psagg bank."""
                vslab, mslab = st
                used = [(b, t) for b in brange
                        for t in range(cums[g][b][wl], cums[g][b][wl + 1])]
                psagg = psC.tile([65, 128], F32)
                if not used:  # no edges: zero the bank
                    nc.tensor.matmul(psagg[:], lhsT=zero128[:, 0:65],
                                     rhs=zero128[:], start=True, stop=True)
                for k, (b, t) in enumerate(used):
                    nc.tensor.matmul(
                        psagg[:], lhsT=vslab[:, b, t, 0:65],
                        rhs=mslab[:, b, :, t],
                        start=(k == 0), stop=(k == len(used) - 1))
                return psagg

            def epilogue(layer, g, aggs):
                # all-DVE epilogue (exp is the only Act op per group, so the
                # in-order Act queue never head-blocks on the div chain)
                bias = b1 if layer == 0 else b2
                den = ep.tile([1, EPW], F32, tag="den")
                nc.scalar.activation(den[:], aggs[64:65, :], Act.Copy,
                                     bias=EPS)
                nc.vector.reciprocal_approx_fast(den[:], den[:])
                ps_rec = psD.tile([D, EPW], F32)
                nc.tensor.matmul(ps_rec[:], lhsT=ones1[:], rhs=den[:],
                                 start=True, stop=True)
                tmp = ep.tile([D, EPW], F32, tag="tmp")
                nc.vector.tensor_tensor(out=tmp[:], in0=aggs[0:64, :],
                                        in1=ps_rec[:], op=Alu.mult)
                w0 = g * GRP
                if layer == 0:
                    nc.scalar.activation(x2T[:, w0 * P:(w0 + GRP) * P],
                                         tmp[:], Act.Relu, bias=bias[:, 0:1])
                    # layer-2 phase A, deferred one group so its x2T input
                    # is long done and PE never stalls at the queue head
                    if g >= 1:
                        wa = (g - 1) * GRP
                        phase_a_grp(1, wa, [x2T[:, (wa + k) * P:
                                                (wa + k + 1) * P]
                                            for k in range(GRP)])
                    if g == NG - 1:
                        wa = g * GRP
                        phase_a_grp(1, wa, [x2T[:, (wa + k) * P:
                                                (wa + k + 1) * P]
                                            for k in range(GRP)])
                else:
                    # write the biased output over aggs (no longer needed)
                    nc.scalar.activation(aggs[0:64, :], tmp[:], Act.Identity,
                                         bias=bias[:, 0:1])
                    nc.sync.dma_start(
                        out_d.ap()[:, w0 * P:(w0 + GRP) * P], aggs[0:64, :])

            def tail0(g, st):
                """Layer-0: score+aggregate all 4 buckets + epilogue."""
                score_v(0, g, st, 0, 4)
                aggs = ep.tile([65, EPW], F32, tag="aggs")
                for wl in range(GRP):
                    w = g * GRP + wl
                    psagg = agg_win(st, cums1, g, wl, range(4))
                    nc.scalar.copy(aggs[:, wl * 128:(wl + 1) * 128],
                                   psagg[:])
                epilogue(0, g, aggs)

            def tailA(g, st):
                """Layer-1 A-half: buckets 0-1 pre-aggregated to a stash."""
                score_v(1, g, st, 0, 2)
                aggs = ep.tile([65, EPW], F32, tag="aggs")
                for wl in range(GRP):
                    w = g * GRP + wl
                    psagg = agg_win(st, cums2, g, wl, range(2))
                    nc.scalar.copy(aggs[:, wl * 128:(wl + 1) * 128],
                                   psagg[:])
                nc.sync.dma_start(stash_d.ap()[g * 65:(g + 1) * 65, :],
                                  aggs[:])

            def tailB(g, st, sta):
                """Layer-1 B-half: buckets 2-3, merge stash, epilogue."""
                score_v(1, g, st, 2, 2)
                aggs = ep.tile([65, EPW], F32, tag="aggs")
                for wl in range(GRP):
                    psagg = agg_win(st, cums2, g, wl, range(2, 4))
                    nc.vector.tensor_tensor(
                        out=aggs[:, wl * 128:(wl + 1) * 128],
                        in0=psagg[:], in1=sta[:, wl * 128:(wl + 1) * 128],
                        op=Alu.add)
                epilogue(1, g, aggs)

            def sweep(layer, dstl, cums, wb, keep=None):
                # a_d expansion sweep, timed to fill the AllGather window:
                # per group, build the one-hot masks once on DVE, transpose
                # used tiles on PE (via the identity permutation), copy the
                # transposed masks to SBUF (DVE/Act alternating), then one
                # near-free PE matvec per tile gives a_d[dst] per edge.
                with tc.tile_wait_until(wb), tc.high_priority():
                    for g in range(NG):
                        c0 = g * NCOLS
                        msw = swp.tile([P, 4, 128, GT], BF, tag="msw")
                        dstl_v = dstl[:, c0:c0 + NCOLS] \
                            .rearrange("p (b t) -> p b t", b=4)
                        nc.vector.tensor_tensor(
                            out=msw[:],
                            in0=iotar_v[:, None, :, :]
                            .to_broadcast([P, 4, 128, GT]),
                            in1=dstl_v[:, :, None, :]
                            .to_broadcast([P, 4, 128, GT]),
                            op=Alu.is_equal)
                        psf = psF.tile([P, NCOLS], F32)
                        cols = []
                        for ci in range(NCOLS):
                            b, t = divmod(ci, GT)
                            c = cums[g][b]
                            if t < c[GRP]:
                                wl = next(i for i in range(GRP)
                                          if c[i] <= t < c[i + 1])
                                cols.append((ci, b, t, g * GRP + wl, True))
                            else:
                                cols.append((ci, b, t, g * GRP, False))
                        usedc = [c for c in cols if c[4]]
                        for j0 in range(0, len(usedc), 8):
                            ch = usedc[j0:j0 + 8]
                            pst = psT.tile([P, 1024], BF)
                            for j, (ci, b, t, w, _) in enumerate(ch):
                                nc.tensor.transpose(
                                    pst[:, j * 128:(j + 1) * 128],
                                    msw[:, b, :, t], identity[:])
                            mss = msp.tile([P, 1024], BF, tag="msS")
                            wid = len(ch) * 128
                            if (j0 // 8) % 2 == 0:
                                nc.vector.tensor_copy(mss[:, 0:wid],
                                                      pst[:, 0:wid])
                            else:
                                nc.scalar.copy(mss[:, 0:wid], pst[:, 0:wid])
                            for j, (ci, b, t, w, _) in enumerate(ch):
                                nc.tensor.matmul(
                                    psf[:, ci:ci + 1],
                                    lhsT=mss[:, j * 128:(j + 1) * 128],
                                    rhs=adcolT[:, w:w + 1],
                                    start=True, stop=True)
                        for (ci, b, t, w, u) in cols:
                            if not u:  # keep padding columns finite (zero)
                                nc.tensor.matmul(
                                    psf[:, ci:ci + 1], lhsT=zero128[:],
                                    rhs=adcolT[:, w:w + 1],
                                    start=True, stop=True)
                        nc.vector.tensor_copy(adpe_d[:, c0:c0 + NCOLS],
                                              psf[:])
                        if keep is not None:
                            keep[g] = msw

            # ================= layer 1 =================
            nc.gpsimd.collective_compute(
                "AllGather", mybir.AluOpType.bypass,
                replica_groups=[list(range(NCORES))],
                ins=[shard1.ap()[:, :]], outs=[tbl1.ap()[:, :]])
            sweep(0, dstl_all, cums1, 0.09)
            ws, dg = 0.36, 0.011
            idxs = {g: load_idx(0, g, 0, 4) for g in range(min(3, NG))}
            with tc.tile_wait_until(ws):
                st = head(0, 0, idxs[0], 0, 4, dstl_all)
            for g in range(NG):
                if g + 3 < NG:
                    with tc.tile_wait_until(ws + g * dg):
                        idxs[g + 3] = load_idx(0, g + 3, 0, 4)
                if g + 1 < NG:
                    with tc.tile_wait_until(ws + (g + 1) * dg):
                        st_next = head(0, g + 1, idxs.pop(g + 1), 0, 4,
                                       dstl_all)
                with tc.tile_wait_until(ws + g * dg + 0.006):
                    tail0(g, st)
                if g + 1 < NG:
                    st = st_next

            # ================= layer 2 =================
            # Table AllGathered in two halves: the A-half collective is
            # hidden under layer-1's edge phase; during the B-half
            # collective the A-bucket edges are gathered, scored and
            # pre-aggregated into a DRAM stash.
            nc.sync.dma_start(dstl_all[:], dstloc2_d.ap()[:, :])
            nc.gpsimd.collective_compute(
                "AllGather", mybir.AluOpType.bypass,
                replica_groups=[list(range(NCORES))],
                ins=[shard2a.ap()[:, :]], outs=[tbl2a.ap()[:, :]])
            nc.gpsimd.collective_compute(
                "AllGather", mybir.AluOpType.bypass,
                replica_groups=[list(range(NCORES))],
                ins=[shard2b.ap()[:, :]], outs=[tbl2b.ap()[:, :]])
            msws = {}
            sweep(1, dstl_all, cums2, 0.80, keep=msws)
            wa, dga = 0.82, 0.0065
            idxsA = {g: load_idx(1, g, 0, 2) for g in range(min(3, NG))}
            with tc.tile_wait_until(wa):
                stA = head(1, 0, idxsA[0], 0, 2, dstl_all,
                           mask=msws.pop(0))
            for g in range(NG):
                if g + 3 < NG:
                    with tc.tile_wait_until(wa + g * dga):
                        idxsA[g + 3] = load_idx(1, g + 3, 0, 2)
                if g + 1 < NG:
                    with tc.tile_wait_until(wa + (g + 1) * dga):
                        stA_next = head(1, g + 1, idxsA.pop(g + 1), 0, 2,
                                        dstl_all, mask=msws.pop(g + 1))
                with tc.tile_wait_until(wa + g * dga + 0.003):
                    tailA(g, stA)
                if g + 1 < NG:
                    stA = stA_next

            def load_stash(g):
                t = ep.tile([65, EPW], F32, tag="stin")
                nc.sync.dma_start(t[:],
                                  stash_d.ap()[g * 65:(g + 1) * 65, :])
                return t

            wsb, dgb = 1.08, 0.008
            idxsB = {g: load_idx(1, g, 2, 2) for g in range(min(3, NG))}
            stashes = {g: load_stash(g) for g in range(min(2, NG))}
            with tc.tile_wait_until(wsb):
                stB = head(1, 0, idxsB[0], 2, 2, dstl_all)
            for g in range(NG):
                if g + 3 < NG:
                    with tc.tile_wait_until(wsb + g * dgb):
                        idxsB[g + 3] = load_idx(1, g + 3, 2, 2)
                if g + 2 < NG:
                    with tc.tile_wait_until(wsb + g * dgb):
                        stashes[g + 2] = load_stash(g + 2)
                if g + 1 < NG:
                    with tc.tile_wait_until(wsb + (g + 1) * dgb):
                        stB_next = head(1, g + 1, idxsB.pop(g + 1), 2, 2,
                                        dstl_all)
                with tc.tile_wait_until(wsb + g * dgb + 0.004):
                    tailB(g, stB, stashes.pop(g))
                if g + 1 < NG:
                    stB = stB_next

    nc.compile()
    return nc


def kernel(x, edge_index, W1, att_src1, att_dst1, b1, W2, att_src2,
           att_dst2, b2):
    from concourse.bass_utils import run_bass_kernel_spmd

    x = np.asarray(x, np.float32)
    edge_index = np.asarray(edge_index)
    W1 = np.asarray(W1, np.float32)
    W2 = np.asarray(W2, np.float32)

    ek = edge_index.tobytes()
    cached = _CACHE.get("pre")
    if cached is not None and cached[0] == ek and \
            np.array_equal(cached[1], x):
        _, _, meta, percore, node_of_slot = cached
    else:
        meta, percore, node_of_slot = _preprocess(x, edge_index)
        _CACHE["pre"] = (ek, x.copy(), meta, percore, node_of_slot)
    mk = tuple(sorted(meta.items()))
    cached = _CACHE.get("prog")
    if cached is not None and cached[0] == mk:
        nc = cached[1]
    else:
        nc = _build_program(meta)
        _CACHE["prog"] = (mk, nc)
    SLOTS_PC, NSLOT, N = meta["SLOTS_PC"], meta["NSLOT"], meta["N"]

    w1cat = np.concatenate([W1, (W1 @ np.asarray(att_src1, np.float32))[:, None]],
                           axis=1).astype(BF16)
    w2cat = np.concatenate([W2, (W2 @ np.asarray(att_src2, np.float32))[:, None]],
                           axis=1).astype(BF16)
    wd1 = np.tile((W1 @ np.asarray(att_dst1, np.float32))[:, None],
                  (1, 128)).astype(BF16)
    wd2 = np.tile((W2 @ np.asarray(att_dst2, np.float32))[:, None],
                  (1, 128)).astype(BF16)
    b1c = np.asarray(b1, np.float32)[:, None]
    b2c = np.asarray(b2, np.float32)[:, None]
    ones1 = np.ones((1, D), np.float32)

    in_maps = []
    for c in range(NCORES):
        in_maps.append({
            "xT": percore["xT"][c], "gidx": percore["gidx"][c],
            "dstloc": percore["dstloc"][c],
            "gidx2": percore["gidx2"][c],
            "dstloc2": percore["dstloc2"][c],
            "w1cat": w1cat, "w2cat": w2cat, "wd1rep": wd1, "wd2rep": wd2,
            "b1": b1c, "b2": b2c, "ones1": ones1,
        })
    res = run_bass_kernel_spmd(nc, in_maps, list(range(NCORES)))

    out = np.empty((N, D), np.float32)
    for c in range(NCORES):
        blk = res.results[c]["out2T"]  # [64, SLOTS_PC]
        sl = node_of_slot[c * SLOTS_PC:(c + 1) * SLOTS_PC]
        valid = sl >= 0
        out[sl[valid]] = blk.T[valid]
    return out

